# revision 9
# baseline (speedup 1.0000x reference)
"""Trainium2 Bass kernel for CausalWaveletFieldAttention.

Shapes (hardcoded): x [B=4, N=4096, D=1024], H=16 heads, HD=64.
Sharding over 8 cores: core c handles (batch b = c//2, half = c%2), i.e.
2048 contiguous sequence rows of one batch.

Per-core pipeline (all big compute in "transposed" [channel, seq] layout):
  1. qkv^T = Wqkv-chunks (stationary) x x^T (moving)  -> [3072, 2048]
  2. k_mag per head via PE block-ones reduction of k^2, f0 = v * k_mag
  3. pairwise AllGather of f0 between the two halves of each batch
     (odd cores need the even core's f0 as causal-conv history; even
     cores multiply the gathered buffer by mask=0)
  4. 24-tap dilated wavelet FIR (softmax gains folded host-side) as PE
     diagonal-matmul PSUM accumulation over shifted slices, extended
     1024 rows back so the d=512/1024 skip taps can be applied locally
  5. skip taps via fused scalar_tensor_tensor on DVE
  6. head coupling as a dense [1024,1024] matmul with (coup^T (x) I_64)
  7. gate = sigmoid(q @ Wgate + b) fused into PSUM eviction on ScalarE
  8. out = (field*gate) @ Wout + bout, stored in natural [seq, ch] layout

All matmuls run in float32r (TF32-like, 4x faster than fp32 on TRN2 PE,
~1.4e-4 rel err per matmul).
"""

import numpy as np

import concourse.bass as bass
import concourse.mybir as mybir
import concourse.tile as tile
from concourse import bacc
from concourse.bass_utils import run_bass_kernel_spmd
from concourse.masks import make_identity

F32 = mybir.dt.float32
F32R = mybir.dt.float32r
AF = mybir.ActivationFunctionType

B, N, D, H, HD = 4, 4096, 1024, 16, 64
NCORES = 8
SEQ = N // 2          # 2048 rows per core
KC = D // 128         # 8 contraction chunks
CONVN = SEQ + 1024    # 3072 conv outputs (1024 extra for skip taps)
EXT = SEQ + 4096      # 6144 extended f0 buffer
D4 = [0.4829629131445341, 0.8365163037378079, 0.2241438680420134, -0.1294095225512604]
N_SCALES = 11
SPARSE_DILATIONS = (512, 1024)
SHIFTS = [0, 1, 2, 3, 4, 6, 8, 12, 16, 24, 32, 48, 64, 96, 128, 192, 256,
          384, 512, 768, 1024, 1536, 2048, 3072]
NT = len(SHIFTS)      # 24 taps

_PROGRAM_CACHE = {}


def _build_program(debug_outputs=False):
    key = bool(debug_outputs)
    if key in _PROGRAM_CACHE:
        return _PROGRAM_CACHE[key]

    nc = bacc.Bacc("TRN2", target_bir_lowering=False, debug=False,
                   num_devices=NCORES)

    # ---- parameters (per-core) ----
    xT = nc.declare_dram_parameter("xT", [D, SEQ], F32, isOutput=False)
    mask = nc.declare_dram_parameter("mask", [128, 1], F32, isOutput=False)
    Wqkv = nc.declare_dram_parameter("Wqkv", [D, 3 * D], F32, isOutput=False)
    bqkvT = nc.declare_dram_parameter("bqkvT", [128, 24], F32, isOutput=False)
    Wgate = nc.declare_dram_parameter("Wgate", [D, D], F32, isOutput=False)
    bgateT = nc.declare_dram_parameter("bgateT", [128, 8], F32, isOutput=False)
    Wout = nc.declare_dram_parameter("Wout", [D, D], F32, isOutput=False)
    boutB = nc.declare_dram_parameter("boutB", [128, D], F32, isOutput=False)
    Mcoup = nc.declare_dram_parameter("Mcoup", [D, D], F32, isOutput=False)
    wchan = nc.declare_dram_parameter("wchan", [128, KC, NT], F32, isOutput=False)
    swt = nc.declare_dram_parameter("swt", [128, 2], F32, isOutput=False)
    bo_in = nc.declare_dram_parameter("bo_in", [128, 2], F32, isOutput=False)
    on_in = nc.declare_dram_parameter("on_in", [2, 128], F32, isOutput=False)
    out = nc.declare_dram_parameter("out", [SEQ, D], F32, isOutput=True)

    dbg = {}
    if debug_outputs:
        for name, shape in (("dbg_f0", [D, SEQ]), ("dbg_conv", [D, CONVN]),
                            ("dbg_field", [D, SEQ]), ("dbg_gate", [D, SEQ])):
            dbg[name] = nc.declare_dram_parameter(name, shape, F32, isOutput=True)

    # ---- internal DRAM ----
    f0_dram = nc.dram_tensor("f0_dram", [D, SEQ], F32)
    f0_gath = nc.dram_tensor("f0_gath", [2, D, SEQ], F32)
    qT_dram = nc.dram_tensor("qT_dram", [D, SEQ], F32R)
    pgT_dram = nc.dram_tensor("pgT_dram", [D, SEQ], F32R)

    with tile.TileContext(nc) as tc:
        with (
            tc.tile_pool(name="psum", bufs=4, space="PSUM") as psp,
            tc.tile_pool(name="psum2", bufs=2, space="PSUM") as psp2,
            tc.tile_pool(name="const", bufs=1) as constp,
        ):
            # ---- constants ----
            ident = constp.tile([128, 128], F32)
            make_identity(nc, ident[:])
            bo_f = constp.tile([128, 2], F32)
            nc.sync.dma_start(bo_f[:], bo_in[:])
            bo_r = constp.tile([128, 2], F32R)
            nc.vector.tensor_copy(bo_r[:], bo_f[:])
            on_f = constp.tile([2, 128], F32)
            nc.sync.dma_start(on_f[:], on_in[:])
            on_r = constp.tile([2, 128], F32R)
            nc.vector.tensor_copy(on_r[:], on_f[:])

            wchan_t = constp.tile([128, KC, NT], F32)
            nc.sync.dma_start(wchan_t[:], wchan[:])
            swt_t = constp.tile([128, 2], F32)
            nc.sync.dma_start(swt_t[:], swt[:])
            bqkv_t = constp.tile([128, 24], F32)
            nc.sync.dma_start(bqkv_t[:], bqkvT[:])
            bgate_t = constp.tile([128, 8], F32)
            nc.sync.dma_start(bgate_t[:], bgateT[:])
            mask_t = constp.tile([128, 1], F32)
            nc.sync.dma_start(mask_t[:], mask[:])

            # ================= phase B/B': qkv =================
            with (
                tc.tile_pool(name="p_xm", bufs=1) as p_xm,
                tc.tile_pool(name="p_bw", bufs=2) as p_bw,
            ):
                xm_r = p_xm.tile([128, KC, SEQ], F32R, tag="xm_r")
                for k in range(KC):
                    xtmp = p_bw.tile([128, SEQ], F32, tag="xtmp")
                    nc.sync.dma_start(xtmp[:], xT[k * 128:(k + 1) * 128, :])
                    nc.vector.tensor_copy(xm_r[:, k, :], xtmp[:])

                def load_strip(col0):
                    sf = p_bw.tile([128, KC, 128], F32, tag="strip_f")
                    nc.sync.dma_start(
                        sf[:],
                        Wqkv[:, col0 * 128:(col0 + 1) * 128]
                        .rearrange("(kc p) m -> p kc m", p=128))
                    sr = p_bw.tile([128, KC, 128], F32R, tag="strip_r")
                    nc.vector.tensor_copy(sr[:], sf[:])
                    return sr

                def qkv_psum(strip, rb, tag="ps"):
                    ps = psp.tile([128, 512], F32, tag=tag)
                    for k in range(KC):
                        nc.tensor.matmul(ps[:], strip[:, k, :],
                                         xm_r[:, k, rb * 512:(rb + 1) * 512],
                                         start=(k == 0), stop=(k == KC - 1))
                    return ps

                for c in range(KC):
                    ks = load_strip(8 + c)
                    k2b = p_bw.tile([128, SEQ], F32R, tag="k2b")
                    for rb in range(4):
                        ps = qkv_psum(ks, rb)
                        nc.scalar.activation(k2b[:, rb * 512:(rb + 1) * 512],
                                             ps[:], AF.Square,
                                             bias=bqkv_t[:, 8 + c:9 + c])
                    vs = load_strip(16 + c)
                    vTb = p_bw.tile([128, SEQ], F32, tag="vTb")
                    for rb in range(4):
                        ps = qkv_psum(vs, rb)
                        nc.scalar.activation(vTb[:, rb * 512:(rb + 1) * 512],
                                             ps[:], AF.Identity,
                                             bias=bqkv_t[:, 16 + c:17 + c])
                    km = p_bw.tile([2, SEQ], F32R, tag="km")
                    for sb in range(4):
                        pss = psp2.tile([2, 512], F32, tag="ps2")
                        nc.tensor.matmul(pss[:], bo_r[:],
                                         k2b[:, sb * 512:(sb + 1) * 512],
                                         start=True, stop=True)
                        nc.scalar.activation(km[:, sb * 512:(sb + 1) * 512],
                                             pss[:], AF.Sqrt)
                    f0b = p_bw.tile([128, SEQ], F32, tag="f0b")
                    for sb in range(4):
                        pse = psp.tile([128, 512], F32, tag="ps")
                        nc.tensor.matmul(pse[:], on_r[:],
                                         km[:, sb * 512:(sb + 1) * 512],
                                         start=True, stop=True)
                        nc.vector.tensor_mul(f0b[:, sb * 512:(sb + 1) * 512],
                                             vTb[:, sb * 512:(sb + 1) * 512],
                                             pse[:])
                    nc.sync.dma_start(f0_dram[c * 128:(c + 1) * 128, :], f0b[:])

                # pairwise f0 exchange
                nc.gpsimd.collective_compute(
                    "AllGather", mybir.AluOpType.bypass,
                    replica_groups=[[0, 1], [2, 3], [4, 5], [6, 7]],
                    ins=[f0_dram[:]], outs=[f0_gath[:]])

                # q columns -> qT_dram
                for o in range(8):
                    qs = load_strip(o)
                    for rb in range(4):
                        ps = qkv_psum(qs, rb)
                        qst = p_bw.tile([128, 512], F32R, tag="qst")
                        nc.vector.tensor_scalar_add(qst[:], ps[:],
                                                    bqkv_t[:, o:o + 1])
                        nc.sync.dma_start(
                            qT_dram[o * 128:(o + 1) * 128,
                                    rb * 512:(rb + 1) * 512], qst[:])

            if debug_outputs:
                nc.sync.dma_start(dbg["dbg_f0"][:], f0_dram[:])

            # ================= phase C..E =================
            with tc.tile_pool(name="p_field", bufs=1) as p_field:
                field = p_field.tile([128, KC, SEQ], F32R, tag="field")

                # ---- conv + skip per chunk ----
                with tc.tile_pool(name="p_cw", bufs=1) as p_cw:
                    ext = p_cw.tile([128, EXT], F32R, tag="ext")
                    zc = p_cw.tile([128, 2048], F32, tag="zc")
                    nc.vector.memset(zc[:], 0.0)
                    nc.vector.tensor_copy(ext[:, 0:2048], zc[:])
                    for c in range(KC):
                        halo = p_cw.tile([128, SEQ], F32, tag="halo")
                        nc.sync.dma_start(halo[:],
                                          f0_gath[0, c * 128:(c + 1) * 128, :])
                        nc.vector.tensor_scalar_mul(ext[:, 2048:4096], halo[:],
                                                    mask_t[:, 0:1])
                        loc = p_cw.tile([128, SEQ], F32, tag="loc")
                        nc.sync.dma_start(loc[:], f0_dram[c * 128:(c + 1) * 128, :])
                        nc.vector.tensor_copy(ext[:, 4096:EXT], loc[:])

                        wd = p_cw.tile([128, NT, 128], F32R, tag="wd")
                        for si in range(NT):
                            nc.vector.tensor_scalar_mul(
                                wd[:, si, :], ident[:],
                                wchan_t[:, c, si:si + 1])
                        convb = p_cw.tile([128, CONVN], F32, tag="convb")
                        for ob in range(CONVN // 512):
                            psc = psp.tile([128, 512], F32, tag="ps")
                            for si, s in enumerate(SHIFTS):
                                off = 3072 + ob * 512 - s
                                nc.tensor.matmul(psc[:], wd[:, si, :],
                                                 ext[:, off:off + 512],
                                                 start=(si == 0),
                                                 stop=(si == NT - 1))
                            nc.vector.tensor_copy(
                                convb[:, ob * 512:(ob + 1) * 512], psc[:])
                        tmp = p_cw.tile([128, SEQ], F32, tag="skiptmp")
                        nc.vector.scalar_tensor_tensor(
                            tmp[:], convb[:, 512:512 + SEQ], swt_t[:, 0:1],
                            convb[:, 1024:1024 + SEQ],
                            op0=mybir.AluOpType.mult, op1=mybir.AluOpType.add)
                        nc.vector.scalar_tensor_tensor(
                            field[:, c, :], convb[:, 0:SEQ], swt_t[:, 1:2],
                            tmp[:],
                            op0=mybir.AluOpType.mult, op1=mybir.AluOpType.add)
                        if debug_outputs:
                            nc.sync.dma_start(
                                dbg["dbg_conv"][c * 128:(c + 1) * 128, :],
                                convb[:])

                # ---- gate (D) then coupling+mult (E) ----
                with tc.tile_pool(name="p_gate", bufs=1) as p_gate:
                    gateT = p_gate.tile([128, 8, SEQ], F32, tag="gateT")
                    with (
                        tc.tile_pool(name="p_wg", bufs=1) as p_wg,
                        tc.tile_pool(name="p_qrb", bufs=1) as p_qrb,
                    ):
                        wg_r = p_wg.tile([128, KC, D], F32R, tag="wg_r")
                        for k in range(KC):
                            wgf = p_qrb.tile([128, D], F32, tag="wgf")
                            nc.sync.dma_start(wgf[:],
                                              Wgate[k * 128:(k + 1) * 128, :])
                            nc.vector.tensor_copy(wg_r[:, k, :], wgf[:])
                        for rb in range(4):
                            qrb = p_qrb.tile([128, KC, 512], F32R, tag="qrb")
                            nc.sync.dma_start(
                                qrb[:],
                                qT_dram[:, rb * 512:(rb + 1) * 512]
                                .rearrange("(kc p) n -> p kc n", p=128))
                            for gc in range(8):
                                psg = psp.tile([128, 512], F32, tag="ps")
                                for k in range(KC):
                                    nc.tensor.matmul(
                                        psg[:],
                                        wg_r[:, k, gc * 128:(gc + 1) * 128],
                                        qrb[:, k, :],
                                        start=(k == 0), stop=(k == KC - 1))
                                nc.scalar.activation(
                                    gateT[:, gc, rb * 512:(rb + 1) * 512],
                                    psg[:], AF.Sigmoid,
                                    bias=bgate_t[:, gc:gc + 1])

                    if debug_outputs:
                        with tc.tile_pool(name="p_dbg", bufs=2) as p_dbg:
                            for c in range(KC):
                                dft = p_dbg.tile([128, SEQ], F32, tag="dft")
                                nc.vector.tensor_copy(dft[:], field[:, c, :])
                                nc.sync.dma_start(
                                    dbg["dbg_field"][c * 128:(c + 1) * 128, :],
                                    dft[:])
                                dgt = p_dbg.tile([128, SEQ], F32, tag="dgt")
                                nc.vector.tensor_copy(dgt[:], gateT[:, c, :])
                                nc.sync.dma_start(
                                    dbg["dbg_gate"][c * 128:(c + 1) * 128, :],
                                    dgt[:])

                    # ---- E: coupling + gate multiply -> pgT_dram ----
                    with tc.tile_pool(name="p_ew", bufs=2) as p_ew:
                        for co in range(KC):
                            mf = p_ew.tile([128, KC, 128], F32, tag="mc_f")
                            nc.sync.dma_start(
                                mf[:],
                                Mcoup[:, co * 128:(co + 1) * 128]
                                .rearrange("(kc p) m -> p kc m", p=128))
                            mr = p_ew.tile([128, KC, 128], F32R, tag="mc_r")
                            nc.vector.tensor_copy(mr[:], mf[:])
                            for sb in range(4):
                                psc2 = psp.tile([128, 512], F32, tag="ps")
                                for ci in range(KC):
                                    nc.tensor.matmul(
                                        psc2[:], mr[:, ci, :],
                                        field[:, ci, sb * 512:(sb + 1) * 512],
                                        start=(ci == 0), stop=(ci == KC - 1))
                                pgs = p_ew.tile([128, 512], F32R, tag="pgs")
                                nc.vector.tensor_mul(
                                    pgs[:], psc2[:],
                                    gateT[:, co, sb * 512:(sb + 1) * 512])
                                nc.sync.dma_start(
                                    pgT_dram[co * 128:(co + 1) * 128,
                                             sb * 512:(sb + 1) * 512], pgs[:])

            # ================= phase F: final projection =================
            with (
                tc.tile_pool(name="p_wo", bufs=1) as p_wo,
                tc.tile_pool(name="p_fw", bufs=2) as p_fw,
            ):
                wo_r = p_wo.tile([128, KC, D], F32R, tag="wo_r")
                for k in range(KC):
                    wof = p_fw.tile([128, D], F32, tag="wof")
                    nc.sync.dma_start(wof[:], Wout[k * 128:(k + 1) * 128, :])
                    nc.vector.tensor_copy(wo_r[:, k, :], wof[:])
                bout_t = p_wo.tile([128, D], F32, tag="bout_t")
                nc.sync.dma_start(bout_t[:], boutB[:])
                for st in range(SEQ // 128):
                    pgt = p_fw.tile([128, KC, 128], F32R, tag="pgt")
                    nc.sync.dma_start(
                        pgt[:],
                        pgT_dram[:, st * 128:(st + 1) * 128]
                        .rearrange("(kc p) m -> p kc m", p=128))
                    outb = p_fw.tile([128, D], F32, tag="outb")
                    for cb in range(2):
                        pso = psp.tile([128, 512], F32, tag="ps")
                        for k in range(KC):
                            nc.tensor.matmul(pso[:], pgt[:, k, :],
                                             wo_r[:, k, cb * 512:(cb + 1) * 512],
                                             start=(k == 0), stop=(k == KC - 1))
                        nc.vector.tensor_add(outb[:, cb * 512:(cb + 1) * 512],
                                             pso[:],
                                             bout_t[:, cb * 512:(cb + 1) * 512])
                    nc.sync.dma_start(out[st * 128:(st + 1) * 128, :], outb[:])

    nc.compile()
    _PROGRAM_CACHE[key] = nc
    return nc


def _softmax(a, axis):
    a = a - a.max(axis=axis, keepdims=True)
    e = np.exp(a)
    return e / e.sum(axis=axis, keepdims=True)


def _host_prep(inputs):
    """Build per-core and replicated input tensors from full inputs."""
    x = np.asarray(inputs["x"], np.float32)
    Wqkv = np.ascontiguousarray(np.asarray(inputs["Wqkv"], np.float32))
    bqkv = np.asarray(inputs["bqkv"], np.float32)
    Wout = np.ascontiguousarray(np.asarray(inputs["Wout"], np.float32))
    bout = np.asarray(inputs["bout"], np.float32)
    Wgate = np.ascontiguousarray(np.asarray(inputs["Wgate"], np.float32))
    bgate = np.asarray(inputs["bgate"], np.float32)
    scale_gain = np.asarray(inputs["scale_gain"], np.float64)
    skip_w = np.asarray(inputs["skip_w"], np.float64)
    coupling = np.asarray(inputs["coupling"], np.float64)

    gains = _softmax(scale_gain, axis=0)              # [11, H]
    sw = 1.0 / (1.0 + np.exp(-skip_w))                # [2]
    coup = _softmax(coupling, axis=-1)                # [H, H]

    sidx = {s: i for i, s in enumerate(SHIFTS)}
    wtab = np.zeros((NT, H), np.float64)
    for j in range(N_SCALES):
        d = 1 << j
        for t in range(4):
            wtab[sidx[(3 - t) * d]] += D4[t] * gains[j]
    ch = np.arange(D)
    wchan = np.zeros((128, KC, NT), np.float32)
    for c in range(KC):
        heads = (ch[c * 128:(c + 1) * 128] // HD)
        wchan[:, c, :] = wtab[:, heads].T.astype(np.float32)

    Mc = np.zeros((D, D), np.float32)
    idx = np.arange(HD)
    for i in range(H):
        for j in range(H):
            Mc[j * HD + idx, i * HD + idx] = coup[i, j]

    bqkvT = bqkv.reshape(24, 128).T.copy()            # [128, 24]
    bgateT = bgate.reshape(8, 128).T.copy()           # [128, 8]
    boutB = np.broadcast_to(bout, (128, D)).copy()
    swt = np.broadcast_to(sw.astype(np.float32), (128, 2)).copy()
    bo = np.zeros((128, 2), np.float32)
    bo[0:64, 0] = 1.0
    bo[64:128, 1] = 1.0
    on = np.zeros((2, 128), np.float32)
    on[0, 0:64] = 1.0
    on[1, 64:128] = 1.0

    shared = dict(Wqkv=Wqkv, bqkvT=bqkvT, Wgate=Wgate, bgateT=bgateT,
                  Wout=Wout, boutB=boutB, Mcoup=Mc, wchan=wchan, swt=swt,
                  bo_in=bo, on_in=on)
    in_maps = []
    for c in range(NCORES):
        b, half = c // 2, c % 2
        g0 = half * SEQ
        xTc = np.ascontiguousarray(x[b, g0:g0 + SEQ, :].T)
        m = np.full((128, 1), float(half), np.float32)
        in_maps.append(dict(xT=xTc, mask=m, **shared))
    return in_maps


def run_cores(inputs, debug_outputs=False, trace=False):
    nc = _build_program(debug_outputs=debug_outputs)
    in_maps = _host_prep(inputs)
    res = run_bass_kernel_spmd(nc, in_maps, list(range(NCORES)), trace=trace)
    return res


def kernel(**inputs) -> np.ndarray:
    res = run_cores(inputs)
    out = np.empty((B, N, D), np.float32)
    for c in range(NCORES):
        b, half = c // 2, c % 2
        out[b, half * SEQ:(half + 1) * SEQ, :] = res.results[c]["out"]
    return out


# revision 15
# speedup vs baseline: 1.2466x; 1.2466x over previous
"""Trainium2 Bass kernel for CausalWaveletFieldAttention.

Shapes (hardcoded): x [B=4, N=4096, D=1024], H=16 heads, HD=64.
Sharding over 8 cores: core c handles (batch b = c//2, half = c%2), i.e.
2048 contiguous sequence rows of one batch.

Per-core pipeline (all big compute in "transposed" [channel, seq] layout):
  1. qkv^T = Wqkv-chunks (stationary) x x^T (moving)  -> [3072, 2048]
  2. k_mag per head via PE block-ones reduction of k^2, f0 = v * k_mag
  3. pairwise AllGather of f0 between the two halves of each batch
     (odd cores need the even core's f0 as causal-conv history; even
     cores multiply the gathered buffer by mask=0)
  4. 24-tap dilated wavelet FIR (softmax gains folded host-side) as PE
     diagonal-matmul PSUM accumulation over shifted slices, extended
     1024 rows back so the d=512/1024 skip taps can be applied locally
  5. skip taps via fused scalar_tensor_tensor on DVE
  6. head coupling as a dense [1024,1024] matmul with (coup^T (x) I_64)
  7. gate = sigmoid(q @ Wgate + b) fused into PSUM eviction on ScalarE
  8. out = (field*gate) @ Wout + bout, stored in natural [seq, ch] layout

All matmuls run in float32r (TF32-like, 4x faster than fp32 on TRN2 PE,
~1.4e-4 rel err per matmul).
"""

import numpy as np

import concourse.bass as bass
import concourse.mybir as mybir
import concourse.tile as tile
from concourse import bacc
from concourse.bass_utils import run_bass_kernel_spmd
from concourse.masks import make_identity

F32 = mybir.dt.float32
F32R = mybir.dt.float32r
AF = mybir.ActivationFunctionType

B, N, D, H, HD = 4, 4096, 1024, 16, 64
NCORES = 8
SEQ = N // 2          # 2048 rows per core
KC = D // 128         # 8 contraction chunks
CONVN = SEQ + 1024    # 3072 conv outputs (1024 extra for skip taps)
EXT = SEQ + 4096      # 6144 extended f0 buffer
D4 = [0.4829629131445341, 0.8365163037378079, 0.2241438680420134, -0.1294095225512604]
N_SCALES = 11
SPARSE_DILATIONS = (512, 1024)
SHIFTS = [0, 1, 2, 3, 4, 6, 8, 12, 16, 24, 32, 48, 64, 96, 128, 192, 256,
          384, 512, 768, 1024, 1536, 2048, 3072]
NT = len(SHIFTS)      # 24 taps

_PROGRAM_CACHE = {}


def _build_program(debug_outputs=False):
    key = bool(debug_outputs)
    if key in _PROGRAM_CACHE:
        return _PROGRAM_CACHE[key]

    nc = bacc.Bacc("TRN2", target_bir_lowering=False, debug=False,
                   num_devices=NCORES)

    # ---- parameters (per-core) ----
    xT = nc.declare_dram_parameter("xT", [D, SEQ], F32, isOutput=False)
    mask = nc.declare_dram_parameter("mask", [128, 1], F32, isOutput=False)
    Wqkv = nc.declare_dram_parameter("Wqkv", [D, 3 * D], F32, isOutput=False)
    bqkvT = nc.declare_dram_parameter("bqkvT", [128, 24], F32, isOutput=False)
    Wgate = nc.declare_dram_parameter("Wgate", [D, D], F32, isOutput=False)
    bgateT = nc.declare_dram_parameter("bgateT", [128, 8], F32, isOutput=False)
    Wout = nc.declare_dram_parameter("Wout", [D, D], F32, isOutput=False)
    boutB = nc.declare_dram_parameter("boutB", [128, D], F32, isOutput=False)
    Mcoup = nc.declare_dram_parameter("Mcoup", [D, D], F32, isOutput=False)
    wchan = nc.declare_dram_parameter("wchan", [128, KC, NT], F32, isOutput=False)
    swt = nc.declare_dram_parameter("swt", [128, 2], F32, isOutput=False)
    bo_in = nc.declare_dram_parameter("bo_in", [128, 2], F32, isOutput=False)
    on_in = nc.declare_dram_parameter("on_in", [2, 128], F32, isOutput=False)
    out = nc.declare_dram_parameter("out", [SEQ, D], F32, isOutput=True)

    dbg = {}
    if debug_outputs:
        for name, shape, dt in (("dbg_f0", [D, SEQ], F32R),
                                ("dbg_conv", [D, CONVN], F32),
                                ("dbg_field", [D, SEQ], F32),
                                ("dbg_gate", [D, SEQ], F32)):
            dbg[name] = nc.declare_dram_parameter(name, shape, dt, isOutput=True)

    # ---- internal DRAM ----
    f0_dram = [nc.dram_tensor(f"f0_dram{c}", [128, SEQ], F32R)
               for c in range(KC)]
    f0_gath = [nc.dram_tensor(f"f0_gath{c}", [2, 128, SEQ], F32R)
               for c in range(KC)]
    qT_dram = nc.dram_tensor("qT_dram", [D, SEQ], F32R)
    pgT_dram = nc.dram_tensor("pgT_dram", [D, SEQ], F32R)

    with tile.TileContext(nc) as tc:
        with (
            tc.tile_pool(name="psum", bufs=6, space="PSUM") as psp,
            tc.tile_pool(name="psum2", bufs=2, space="PSUM") as psp2,
            tc.tile_pool(name="const", bufs=1) as constp,
        ):
            # ---- constants ----
            ident = constp.tile([128, 128], F32)
            make_identity(nc, ident[:])
            bo_f = constp.tile([128, 2], F32)
            nc.sync.dma_start(bo_f[:], bo_in[:])
            bo_r = constp.tile([128, 2], F32R)
            nc.vector.tensor_copy(bo_r[:], bo_f[:])
            on_f = constp.tile([2, 128], F32)
            nc.sync.dma_start(on_f[:], on_in[:])
            on_r = constp.tile([2, 128], F32R)
            nc.vector.tensor_copy(on_r[:], on_f[:])

            wchan_t = constp.tile([128, KC, NT], F32)
            nc.sync.dma_start(wchan_t[:], wchan[:])
            swt_t = constp.tile([128, 2], F32)
            nc.sync.dma_start(swt_t[:], swt[:])
            bqkv_t = constp.tile([128, 24], F32)
            nc.sync.dma_start(bqkv_t[:], bqkvT[:])
            bgate_t = constp.tile([128, 8], F32)
            nc.sync.dma_start(bgate_t[:], bgateT[:])
            mask_t = constp.tile([128, 1], F32)
            nc.sync.dma_start(mask_t[:], mask[:])

            # ================= phase B/B': qkv =================
            with (
                tc.tile_pool(name="p_xm", bufs=1) as p_xm,
                tc.tile_pool(name="p_bw", bufs=2) as p_bw,
            ):
                xm_r = p_xm.tile([128, KC, SEQ], F32R, tag="xm_r")
                for k in range(KC):
                    xtmp = p_bw.tile([128, SEQ], F32, tag="xtmp")
                    nc.sync.dma_start(xtmp[:], xT[k * 128:(k + 1) * 128, :])
                    nc.vector.tensor_copy(xm_r[:, k, :], xtmp[:])

                def load_strip(col0):
                    sf = p_bw.tile([128, KC, 128], F32, tag="strip_f")
                    nc.sync.dma_start(
                        sf[:],
                        Wqkv[:, col0 * 128:(col0 + 1) * 128]
                        .rearrange("(kc p) m -> p kc m", p=128))
                    sr = p_bw.tile([128, KC, 128], F32R, tag="strip_r")
                    nc.vector.tensor_copy(sr[:], sf[:])
                    return sr

                def qkv_psum(strip, rb, tag="ps"):
                    ps = psp.tile([128, 512], F32, tag=tag)
                    for k in range(KC):
                        nc.tensor.matmul(ps[:], strip[:, k, :],
                                         xm_r[:, k, rb * 512:(rb + 1) * 512],
                                         start=(k == 0), stop=(k == KC - 1))
                    return ps

                for c in range(KC):
                    ks = load_strip(8 + c)
                    k2b = p_bw.tile([128, SEQ], F32R, tag="k2b")
                    for rb in range(4):
                        ps = qkv_psum(ks, rb)
                        nc.scalar.activation(k2b[:, rb * 512:(rb + 1) * 512],
                                             ps[:], AF.Square,
                                             bias=bqkv_t[:, 8 + c:9 + c])
                    vs = load_strip(16 + c)
                    vTb = p_bw.tile([128, SEQ], F32, tag="vTb")
                    for rb in range(4):
                        ps = qkv_psum(vs, rb)
                        nc.scalar.activation(vTb[:, rb * 512:(rb + 1) * 512],
                                             ps[:], AF.Identity,
                                             bias=bqkv_t[:, 16 + c:17 + c])
                    km = p_bw.tile([2, SEQ], F32R, tag="km")
                    for sb in range(4):
                        pss = psp2.tile([2, 512], F32, tag="ps2")
                        nc.tensor.matmul(pss[:], bo_r[:],
                                         k2b[:, sb * 512:(sb + 1) * 512],
                                         start=True, stop=True)
                        nc.scalar.activation(km[:, sb * 512:(sb + 1) * 512],
                                             pss[:], AF.Sqrt)
                    f0b = p_bw.tile([128, SEQ], F32R, tag="f0b")
                    for sb in range(4):
                        pse = psp.tile([128, 512], F32, tag="ps")
                        nc.tensor.matmul(pse[:], on_r[:],
                                         km[:, sb * 512:(sb + 1) * 512],
                                         start=True, stop=True)
                        nc.vector.tensor_mul(f0b[:, sb * 512:(sb + 1) * 512],
                                             vTb[:, sb * 512:(sb + 1) * 512],
                                             pse[:])
                    nc.sync.dma_start(f0_dram[c][:], f0b[:])
                    # pipelined pairwise f0 exchange (per chunk, overlaps B)
                    nc.gpsimd.collective_compute(
                        "AllGather", mybir.AluOpType.bypass,
                        replica_groups=[[0, 1], [2, 3], [4, 5], [6, 7]],
                        ins=[f0_dram[c][:]], outs=[f0_gath[c][:]])

                # q columns -> qT_dram
                for o in range(8):
                    qs = load_strip(o)
                    for rb in range(4):
                        ps = qkv_psum(qs, rb)
                        qst = p_bw.tile([128, 512], F32R, tag="qst")
                        nc.vector.tensor_scalar_add(qst[:], ps[:],
                                                    bqkv_t[:, o:o + 1])
                        nc.sync.dma_start(
                            qT_dram[o * 128:(o + 1) * 128,
                                    rb * 512:(rb + 1) * 512], qst[:])

            if debug_outputs:
                for c in range(KC):
                    nc.sync.dma_start(
                        dbg["dbg_f0"][c * 128:(c + 1) * 128, :], f0_dram[c][:])

            # ================= phase C..E =================
            with tc.tile_pool(name="p_field", bufs=1) as p_field:
                field = p_field.tile([128, KC, SEQ], F32R, tag="field")

                # ---- conv + skip per chunk ----
                with (
                    tc.tile_pool(name="p_cw", bufs=2) as p_cw,
                    tc.tile_pool(name="p_cw1", bufs=1) as p_cw1,
                ):
                    zc = p_cw1.tile([128, 512], F32, tag="zc")
                    nc.vector.memset(zc[:], 0.0)
                    exts = []
                    for i in range(2):
                        e = p_cw1.tile([128, EXT], F32R, tag=f"ext{i}")
                        for z in range(4):
                            nc.vector.tensor_copy(e[:, z * 512:(z + 1) * 512],
                                                  zc[:])
                        exts.append(e)
                    for c in range(KC):
                        ext = exts[c % 2]
                        halo = p_cw.tile([128, SEQ], F32R, tag="halo")
                        nc.sync.dma_start(halo[:], f0_gath[c][0, :, :])
                        nc.vector.tensor_scalar_mul(ext[:, 2048:4096], halo[:],
                                                    mask_t[:, 0:1])
                        nc.sync.dma_start(ext[:, 4096:EXT], f0_dram[c][:])

                        wd = p_cw.tile([128, NT, 128], F32R, tag="wd")
                        for si in range(NT):
                            nc.vector.tensor_scalar_mul(
                                wd[:, si, :], ident[:],
                                wchan_t[:, c, si:si + 1])
                        convb = p_cw.tile([128, CONVN], F32, tag="convb")
                        for ob in range(CONVN // 512):
                            psc = psp.tile([128, 512], F32, tag="ps")
                            for si, s in enumerate(SHIFTS):
                                off = 3072 + ob * 512 - s
                                nc.tensor.matmul(psc[:], wd[:, si, :],
                                                 ext[:, off:off + 512],
                                                 start=(si == 0),
                                                 stop=(si == NT - 1))
                            nc.vector.tensor_copy(
                                convb[:, ob * 512:(ob + 1) * 512], psc[:])
                        tmp = p_cw.tile([128, SEQ], F32, tag="skiptmp")
                        nc.vector.scalar_tensor_tensor(
                            tmp[:], convb[:, 512:512 + SEQ], swt_t[:, 0:1],
                            convb[:, 1024:1024 + SEQ],
                            op0=mybir.AluOpType.mult, op1=mybir.AluOpType.add)
                        nc.vector.scalar_tensor_tensor(
                            field[:, c, :], convb[:, 0:SEQ], swt_t[:, 1:2],
                            tmp[:],
                            op0=mybir.AluOpType.mult, op1=mybir.AluOpType.add)
                        if debug_outputs:
                            nc.sync.dma_start(
                                dbg["dbg_conv"][c * 128:(c + 1) * 128, :],
                                convb[:])

                # ---- gate (D) then coupling+mult (E) ----
                with tc.tile_pool(name="p_gate", bufs=1) as p_gate:
                    gateT = p_gate.tile([128, 8, SEQ], F32, tag="gateT")
                    with (
                        tc.tile_pool(name="p_wg", bufs=1) as p_wg,
                        tc.tile_pool(name="p_qrb", bufs=1) as p_qrb,
                    ):
                        wg_r = p_wg.tile([128, KC, D], F32R, tag="wg_r")
                        for k in range(KC):
                            wgf = p_qrb.tile([128, D], F32, tag="wgf")
                            nc.sync.dma_start(wgf[:],
                                              Wgate[k * 128:(k + 1) * 128, :])
                            nc.vector.tensor_copy(wg_r[:, k, :], wgf[:])
                        for rb in range(4):
                            qrb = p_qrb.tile([128, KC, 512], F32R, tag="qrb")
                            nc.sync.dma_start(
                                qrb[:],
                                qT_dram[:, rb * 512:(rb + 1) * 512]
                                .rearrange("(kc p) n -> p kc n", p=128))
                            for gc in range(8):
                                psg = psp.tile([128, 512], F32, tag="ps")
                                for k in range(KC):
                                    nc.tensor.matmul(
                                        psg[:],
                                        wg_r[:, k, gc * 128:(gc + 1) * 128],
                                        qrb[:, k, :],
                                        start=(k == 0), stop=(k == KC - 1))
                                nc.scalar.activation(
                                    gateT[:, gc, rb * 512:(rb + 1) * 512],
                                    psg[:], AF.Sigmoid,
                                    bias=bgate_t[:, gc:gc + 1])

                    if debug_outputs:
                        with tc.tile_pool(name="p_dbg", bufs=2) as p_dbg:
                            for c in range(KC):
                                dft = p_dbg.tile([128, SEQ], F32, tag="dft")
                                nc.vector.tensor_copy(dft[:], field[:, c, :])
                                nc.sync.dma_start(
                                    dbg["dbg_field"][c * 128:(c + 1) * 128, :],
                                    dft[:])
                                dgt = p_dbg.tile([128, SEQ], F32, tag="dgt")
                                nc.vector.tensor_copy(dgt[:], gateT[:, c, :])
                                nc.sync.dma_start(
                                    dbg["dbg_gate"][c * 128:(c + 1) * 128, :],
                                    dgt[:])

                    # ---- E: coupling + gate multiply -> pgT_dram ----
                    with tc.tile_pool(name="p_ew", bufs=2) as p_ew:
                        for co in range(KC):
                            mf = p_ew.tile([128, KC, 128], F32, tag="mc_f")
                            nc.sync.dma_start(
                                mf[:],
                                Mcoup[:, co * 128:(co + 1) * 128]
                                .rearrange("(kc p) m -> p kc m", p=128))
                            mr = p_ew.tile([128, KC, 128], F32R, tag="mc_r")
                            nc.vector.tensor_copy(mr[:], mf[:])
                            for sb in range(4):
                                psc2 = psp.tile([128, 512], F32, tag="ps")
                                for ci in range(KC):
                                    nc.tensor.matmul(
                                        psc2[:], mr[:, ci, :],
                                        field[:, ci, sb * 512:(sb + 1) * 512],
                                        start=(ci == 0), stop=(ci == KC - 1))
                                pgs = p_ew.tile([128, 512], F32R, tag="pgs")
                                nc.vector.tensor_mul(
                                    pgs[:], psc2[:],
                                    gateT[:, co, sb * 512:(sb + 1) * 512])
                                nc.sync.dma_start(
                                    pgT_dram[co * 128:(co + 1) * 128,
                                             sb * 512:(sb + 1) * 512], pgs[:])

            # ================= phase F: final projection =================
            with (
                tc.tile_pool(name="p_wo", bufs=1) as p_wo,
                tc.tile_pool(name="p_fw", bufs=2) as p_fw,
            ):
                wo_r = p_wo.tile([128, KC, D], F32R, tag="wo_r")
                for k in range(KC):
                    wof = p_fw.tile([128, D], F32, tag="wof")
                    nc.sync.dma_start(wof[:], Wout[k * 128:(k + 1) * 128, :])
                    nc.vector.tensor_copy(wo_r[:, k, :], wof[:])
                bout_t = p_wo.tile([128, D], F32, tag="bout_t")
                nc.sync.dma_start(bout_t[:], boutB[:])
                for st in range(SEQ // 128):
                    pgt = p_fw.tile([128, KC, 128], F32R, tag="pgt")
                    nc.sync.dma_start(
                        pgt[:],
                        pgT_dram[:, st * 128:(st + 1) * 128]
                        .rearrange("(kc p) m -> p kc m", p=128))
                    outb = p_fw.tile([128, D], F32, tag="outb")
                    for cb in range(2):
                        pso = psp.tile([128, 512], F32, tag="ps")
                        for k in range(KC):
                            nc.tensor.matmul(pso[:], pgt[:, k, :],
                                             wo_r[:, k, cb * 512:(cb + 1) * 512],
                                             start=(k == 0), stop=(k == KC - 1))
                        nc.vector.tensor_add(outb[:, cb * 512:(cb + 1) * 512],
                                             pso[:],
                                             bout_t[:, cb * 512:(cb + 1) * 512])
                    nc.sync.dma_start(out[st * 128:(st + 1) * 128, :], outb[:])

    nc.compile()
    _PROGRAM_CACHE[key] = nc
    return nc


def _softmax(a, axis):
    a = a - a.max(axis=axis, keepdims=True)
    e = np.exp(a)
    return e / e.sum(axis=axis, keepdims=True)


def _host_prep(inputs):
    """Build per-core and replicated input tensors from full inputs."""
    x = np.asarray(inputs["x"], np.float32)
    Wqkv = np.ascontiguousarray(np.asarray(inputs["Wqkv"], np.float32))
    bqkv = np.asarray(inputs["bqkv"], np.float32)
    Wout = np.ascontiguousarray(np.asarray(inputs["Wout"], np.float32))
    bout = np.asarray(inputs["bout"], np.float32)
    Wgate = np.ascontiguousarray(np.asarray(inputs["Wgate"], np.float32))
    bgate = np.asarray(inputs["bgate"], np.float32)
    scale_gain = np.asarray(inputs["scale_gain"], np.float64)
    skip_w = np.asarray(inputs["skip_w"], np.float64)
    coupling = np.asarray(inputs["coupling"], np.float64)

    gains = _softmax(scale_gain, axis=0)              # [11, H]
    sw = 1.0 / (1.0 + np.exp(-skip_w))                # [2]
    coup = _softmax(coupling, axis=-1)                # [H, H]

    sidx = {s: i for i, s in enumerate(SHIFTS)}
    wtab = np.zeros((NT, H), np.float64)
    for j in range(N_SCALES):
        d = 1 << j
        for t in range(4):
            wtab[sidx[(3 - t) * d]] += D4[t] * gains[j]
    ch = np.arange(D)
    wchan = np.zeros((128, KC, NT), np.float32)
    for c in range(KC):
        heads = (ch[c * 128:(c + 1) * 128] // HD)
        wchan[:, c, :] = wtab[:, heads].T.astype(np.float32)

    Mc = np.zeros((D, D), np.float32)
    idx = np.arange(HD)
    for i in range(H):
        for j in range(H):
            Mc[j * HD + idx, i * HD + idx] = coup[i, j]

    bqkvT = bqkv.reshape(24, 128).T.copy()            # [128, 24]
    bgateT = bgate.reshape(8, 128).T.copy()           # [128, 8]
    boutB = np.broadcast_to(bout, (128, D)).copy()
    swt = np.broadcast_to(sw.astype(np.float32), (128, 2)).copy()
    bo = np.zeros((128, 2), np.float32)
    bo[0:64, 0] = 1.0
    bo[64:128, 1] = 1.0
    on = np.zeros((2, 128), np.float32)
    on[0, 0:64] = 1.0
    on[1, 64:128] = 1.0

    shared = dict(Wqkv=Wqkv, bqkvT=bqkvT, Wgate=Wgate, bgateT=bgateT,
                  Wout=Wout, boutB=boutB, Mcoup=Mc, wchan=wchan, swt=swt,
                  bo_in=bo, on_in=on)
    in_maps = []
    for c in range(NCORES):
        b, half = c // 2, c % 2
        g0 = half * SEQ
        xTc = np.ascontiguousarray(x[b, g0:g0 + SEQ, :].T)
        m = np.full((128, 1), float(half), np.float32)
        in_maps.append(dict(xT=xTc, mask=m, **shared))
    return in_maps


def run_cores(inputs, debug_outputs=False, trace=False):
    nc = _build_program(debug_outputs=debug_outputs)
    in_maps = _host_prep(inputs)
    res = run_bass_kernel_spmd(nc, in_maps, list(range(NCORES)), trace=trace)
    return res


def kernel(**inputs) -> np.ndarray:
    res = run_cores(inputs)
    out = np.empty((B, N, D), np.float32)
    for c in range(NCORES):
        b, half = c // 2, c % 2
        out[b, half * SEQ:(half + 1) * SEQ, :] = res.results[c]["out"]
    return out


# revision 16
# speedup vs baseline: 1.2900x; 1.0348x over previous
"""Trainium2 Bass kernel for CausalWaveletFieldAttention.

Shapes (hardcoded): x [B=4, N=4096, D=1024], H=16 heads, HD=64.
Sharding over 8 cores: core c handles (batch b = c//2, half = c%2), i.e.
2048 contiguous sequence rows of one batch.

Per-core pipeline (all big compute in "transposed" [channel, seq] layout):
  1. qkv^T = Wqkv-chunks (stationary) x x^T (moving)  -> [3072, 2048]
  2. k_mag per head via PE block-ones reduction of k^2, f0 = v * k_mag
  3. pairwise AllGather of f0 between the two halves of each batch
     (odd cores need the even core's f0 as causal-conv history; even
     cores multiply the gathered buffer by mask=0)
  4. 24-tap dilated wavelet FIR (softmax gains folded host-side) as PE
     diagonal-matmul PSUM accumulation over shifted slices, extended
     1024 rows back so the d=512/1024 skip taps can be applied locally
  5. skip taps via fused scalar_tensor_tensor on DVE
  6. head coupling as a dense [1024,1024] matmul with (coup^T (x) I_64)
  7. gate = sigmoid(q @ Wgate + b) fused into PSUM eviction on ScalarE
  8. out = (field*gate) @ Wout + bout, stored in natural [seq, ch] layout

All matmuls run in float32r (TF32-like, 4x faster than fp32 on TRN2 PE,
~1.4e-4 rel err per matmul).
"""

import numpy as np

import concourse.bass as bass
import concourse.mybir as mybir
import concourse.tile as tile
from concourse import bacc
from concourse.bass_utils import run_bass_kernel_spmd
from concourse.masks import make_identity

F32 = mybir.dt.float32
F32R = mybir.dt.float32r
AF = mybir.ActivationFunctionType

B, N, D, H, HD = 4, 4096, 1024, 16, 64
NCORES = 8
SEQ = N // 2          # 2048 rows per core
KC = D // 128         # 8 contraction chunks
CONVN = SEQ + 1024    # 3072 conv outputs (1024 extra for skip taps)
EXT = SEQ + 4096      # 6144 extended f0 buffer
D4 = [0.4829629131445341, 0.8365163037378079, 0.2241438680420134, -0.1294095225512604]
N_SCALES = 11
SPARSE_DILATIONS = (512, 1024)
SHIFTS = [0, 1, 2, 3, 4, 6, 8, 12, 16, 24, 32, 48, 64, 96, 128, 192, 256,
          384, 512, 768, 1024, 1536, 2048, 3072]
NT = len(SHIFTS)      # 24 taps

_PROGRAM_CACHE = {}


def _build_program(debug_outputs=False):
    key = bool(debug_outputs)
    if key in _PROGRAM_CACHE:
        return _PROGRAM_CACHE[key]

    nc = bacc.Bacc("TRN2", target_bir_lowering=False, debug=False,
                   num_devices=NCORES)

    # ---- parameters (per-core) ----
    xT = nc.declare_dram_parameter("xT", [D, SEQ], F32, isOutput=False)
    mask = nc.declare_dram_parameter("mask", [128, 1], F32, isOutput=False)
    Wqkv = nc.declare_dram_parameter("Wqkv", [D, 3 * D], F32, isOutput=False)
    bqkvT = nc.declare_dram_parameter("bqkvT", [128, 24], F32, isOutput=False)
    Wgate = nc.declare_dram_parameter("Wgate", [D, D], F32, isOutput=False)
    bgateT = nc.declare_dram_parameter("bgateT", [128, 8], F32, isOutput=False)
    Wout = nc.declare_dram_parameter("Wout", [D, D], F32, isOutput=False)
    boutB = nc.declare_dram_parameter("boutB", [128, D], F32, isOutput=False)
    Mcoup = nc.declare_dram_parameter("Mcoup", [D, D], F32, isOutput=False)
    wchan = nc.declare_dram_parameter("wchan", [128, KC, NT], F32, isOutput=False)
    swt = nc.declare_dram_parameter("swt", [128, 2], F32, isOutput=False)
    bo_in = nc.declare_dram_parameter("bo_in", [128, 2], F32, isOutput=False)
    on_in = nc.declare_dram_parameter("on_in", [2, 128], F32, isOutput=False)
    out = nc.declare_dram_parameter("out", [SEQ, D], F32, isOutput=True)

    dbg = {}
    if debug_outputs:
        for name, shape, dt in (("dbg_f0", [D, SEQ], F32R),
                                ("dbg_conv", [D, CONVN], F32),
                                ("dbg_field", [D, SEQ], F32),
                                ("dbg_gate", [D, SEQ], F32)):
            dbg[name] = nc.declare_dram_parameter(name, shape, dt, isOutput=True)

    # ---- internal DRAM ----
    f0_dram = [nc.dram_tensor(f"f0_dram{c}", [128, SEQ], F32R)
               for c in range(KC)]
    f0_gath = [nc.dram_tensor(f"f0_gath{c}", [2, 128, SEQ], F32R)
               for c in range(KC)]
    qT_dram = nc.dram_tensor("qT_dram", [D, SEQ], F32R)
    pgT_dram = nc.dram_tensor("pgT_dram", [D, SEQ], F32R)

    with tile.TileContext(nc) as tc:
        with (
            tc.tile_pool(name="psum", bufs=6, space="PSUM") as psp,
            tc.tile_pool(name="psum2", bufs=2, space="PSUM") as psp2,
            tc.tile_pool(name="const", bufs=1) as constp,
        ):
            # ---- constants ----
            ident = constp.tile([128, 128], F32)
            make_identity(nc, ident[:])
            bo_f = constp.tile([128, 2], F32)
            nc.sync.dma_start(bo_f[:], bo_in[:])
            bo_r = constp.tile([128, 2], F32R)
            nc.vector.tensor_copy(bo_r[:], bo_f[:])
            on_f = constp.tile([2, 128], F32)
            nc.sync.dma_start(on_f[:], on_in[:])
            on_r = constp.tile([2, 128], F32R)
            nc.vector.tensor_copy(on_r[:], on_f[:])

            wchan_t = constp.tile([128, KC, NT], F32)
            nc.sync.dma_start(wchan_t[:], wchan[:])
            swt_t = constp.tile([128, 2], F32)
            nc.sync.dma_start(swt_t[:], swt[:])
            bqkv_t = constp.tile([128, 24], F32)
            nc.sync.dma_start(bqkv_t[:], bqkvT[:])
            bgate_t = constp.tile([128, 8], F32)
            nc.sync.dma_start(bgate_t[:], bgateT[:])
            mask_t = constp.tile([128, 1], F32)
            nc.sync.dma_start(mask_t[:], mask[:])

            # ================= phase B/B': qkv =================
            with (
                tc.tile_pool(name="p_xm", bufs=1) as p_xm,
                tc.tile_pool(name="p_bw", bufs=2) as p_bw,
            ):
                xm_r = p_xm.tile([128, KC, SEQ], F32R, tag="xm_r")
                for k in range(KC):
                    xtmp = p_bw.tile([128, SEQ], F32, tag="xtmp")
                    nc.sync.dma_start(xtmp[:], xT[k * 128:(k + 1) * 128, :])
                    nc.vector.tensor_copy(xm_r[:, k, :], xtmp[:])

                def load_strip(col0):
                    sf = p_bw.tile([128, KC, 128], F32, tag="strip_f")
                    nc.sync.dma_start(
                        sf[:],
                        Wqkv[:, col0 * 128:(col0 + 1) * 128]
                        .rearrange("(kc p) m -> p kc m", p=128))
                    sr = p_bw.tile([128, KC, 128], F32R, tag="strip_r")
                    nc.vector.tensor_copy(sr[:], sf[:])
                    return sr

                def qkv_psum(strip, rb, tag="ps"):
                    ps = psp.tile([128, 512], F32, tag=tag)
                    for k in range(KC):
                        nc.tensor.matmul(ps[:], strip[:, k, :],
                                         xm_r[:, k, rb * 512:(rb + 1) * 512],
                                         start=(k == 0), stop=(k == KC - 1))
                    return ps

                for c in range(KC):
                    ks = load_strip(8 + c)
                    k2b = p_bw.tile([128, SEQ], F32R, tag="k2b")
                    for rb in range(4):
                        ps = qkv_psum(ks, rb)
                        nc.scalar.activation(k2b[:, rb * 512:(rb + 1) * 512],
                                             ps[:], AF.Square,
                                             bias=bqkv_t[:, 8 + c:9 + c])
                    vs = load_strip(16 + c)
                    vTb = p_bw.tile([128, SEQ], F32, tag="vTb")
                    for rb in range(4):
                        ps = qkv_psum(vs, rb)
                        nc.scalar.activation(vTb[:, rb * 512:(rb + 1) * 512],
                                             ps[:], AF.Identity,
                                             bias=bqkv_t[:, 16 + c:17 + c])
                    km = p_bw.tile([2, SEQ], F32R, tag="km")
                    for sb in range(4):
                        pss = psp2.tile([2, 512], F32, tag="ps2")
                        nc.tensor.matmul(pss[:], bo_r[:],
                                         k2b[:, sb * 512:(sb + 1) * 512],
                                         start=True, stop=True)
                        nc.scalar.activation(km[:, sb * 512:(sb + 1) * 512],
                                             pss[:], AF.Sqrt)
                    f0b = p_bw.tile([128, SEQ], F32R, tag="f0b")
                    for sb in range(4):
                        pse = psp.tile([128, 512], F32, tag="ps")
                        nc.tensor.matmul(pse[:], on_r[:],
                                         km[:, sb * 512:(sb + 1) * 512],
                                         start=True, stop=True)
                        nc.vector.tensor_mul(f0b[:, sb * 512:(sb + 1) * 512],
                                             vTb[:, sb * 512:(sb + 1) * 512],
                                             pse[:])
                    nc.sync.dma_start(f0_dram[c][:], f0b[:])
                    # pipelined pairwise f0 exchange (per chunk, overlaps B)
                    nc.gpsimd.collective_compute(
                        "AllGather", mybir.AluOpType.bypass,
                        replica_groups=[[0, 1], [2, 3], [4, 5], [6, 7]],
                        ins=[f0_dram[c][:]], outs=[f0_gath[c][:]])

                # q columns -> qT_dram
                for o in range(8):
                    qs = load_strip(o)
                    for rb in range(4):
                        ps = qkv_psum(qs, rb)
                        qst = p_bw.tile([128, 512], F32R, tag="qst")
                        nc.vector.tensor_scalar_add(qst[:], ps[:],
                                                    bqkv_t[:, o:o + 1])
                        nc.sync.dma_start(
                            qT_dram[o * 128:(o + 1) * 128,
                                    rb * 512:(rb + 1) * 512], qst[:])

            if debug_outputs:
                for c in range(KC):
                    nc.sync.dma_start(
                        dbg["dbg_f0"][c * 128:(c + 1) * 128, :], f0_dram[c][:])

            # ================= phase C..E =================
            with tc.tile_pool(name="p_field", bufs=1) as p_field:
                field = p_field.tile([128, KC, SEQ], F32R, tag="field")

                # ---- conv + skip per chunk ----
                with (
                    tc.tile_pool(name="p_cw", bufs=2) as p_cw,
                    tc.tile_pool(name="p_cw1", bufs=1) as p_cw1,
                ):
                    zc = p_cw1.tile([128, 512], F32, tag="zc")
                    nc.vector.memset(zc[:], 0.0)
                    exts = []
                    for i in range(2):
                        e = p_cw1.tile([128, EXT], F32R, tag=f"ext{i}")
                        for z in range(4):
                            nc.vector.tensor_copy(e[:, z * 512:(z + 1) * 512],
                                                  zc[:])
                        exts.append(e)
                    for c in range(KC):
                        ext = exts[c % 2]
                        halo = p_cw.tile([128, SEQ], F32R, tag="halo")
                        nc.sync.dma_start(halo[:], f0_gath[c][0, :, :])
                        nc.vector.tensor_scalar_mul(ext[:, 2048:4096], halo[:],
                                                    mask_t[:, 0:1])
                        nc.sync.dma_start(ext[:, 4096:EXT], f0_dram[c][:])

                        wd = p_cw.tile([128, NT, 128], F32R, tag="wd")
                        for si in range(NT):
                            nc.vector.tensor_scalar_mul(
                                wd[:, si, :], ident[:],
                                wchan_t[:, c, si:si + 1])
                        convb = p_cw.tile([128, CONVN], F32, tag="convb")
                        for ob in range(CONVN // 512):
                            psc = psp.tile([128, 512], F32, tag="ps")
                            for si, s in enumerate(SHIFTS):
                                off = 3072 + ob * 512 - s
                                nc.tensor.matmul(psc[:], wd[:, si, :],
                                                 ext[:, off:off + 512],
                                                 start=(si == 0),
                                                 stop=(si == NT - 1))
                            nc.vector.tensor_copy(
                                convb[:, ob * 512:(ob + 1) * 512], psc[:])
                        tmp = p_cw.tile([128, SEQ], F32, tag="skiptmp")
                        nc.vector.scalar_tensor_tensor(
                            tmp[:], convb[:, 512:512 + SEQ], swt_t[:, 0:1],
                            convb[:, 1024:1024 + SEQ],
                            op0=mybir.AluOpType.mult, op1=mybir.AluOpType.add)
                        nc.vector.scalar_tensor_tensor(
                            field[:, c, :], convb[:, 0:SEQ], swt_t[:, 1:2],
                            tmp[:],
                            op0=mybir.AluOpType.mult, op1=mybir.AluOpType.add)
                        if debug_outputs:
                            nc.sync.dma_start(
                                dbg["dbg_conv"][c * 128:(c + 1) * 128, :],
                                convb[:])

                # ---- gate (D) then coupling+mult (E) ----
                with tc.tile_pool(name="p_gate", bufs=1) as p_gate:
                    gateT = p_gate.tile([128, 8, SEQ], F32, tag="gateT")
                    with (
                        tc.tile_pool(name="p_wg", bufs=1) as p_wg,
                        tc.tile_pool(name="p_qrb", bufs=2) as p_qrb,
                    ):
                        wg_r = p_wg.tile([128, KC, D], F32R, tag="wg_r")
                        for k in range(KC):
                            wgf = p_qrb.tile([128, D], F32, tag="wgf")
                            nc.sync.dma_start(wgf[:],
                                              Wgate[k * 128:(k + 1) * 128, :])
                            nc.vector.tensor_copy(wg_r[:, k, :], wgf[:])
                        for rb in range(4):
                            qrb = p_qrb.tile([128, KC, 512], F32R, tag="qrb")
                            nc.sync.dma_start(
                                qrb[:],
                                qT_dram[:, rb * 512:(rb + 1) * 512]
                                .rearrange("(kc p) n -> p kc n", p=128))
                            for gc in range(8):
                                psg = psp.tile([128, 512], F32, tag="ps")
                                for k in range(KC):
                                    nc.tensor.matmul(
                                        psg[:],
                                        wg_r[:, k, gc * 128:(gc + 1) * 128],
                                        qrb[:, k, :],
                                        start=(k == 0), stop=(k == KC - 1))
                                nc.scalar.activation(
                                    gateT[:, gc, rb * 512:(rb + 1) * 512],
                                    psg[:], AF.Sigmoid,
                                    bias=bgate_t[:, gc:gc + 1])

                    if debug_outputs:
                        with tc.tile_pool(name="p_dbg", bufs=2) as p_dbg:
                            for c in range(KC):
                                dft = p_dbg.tile([128, SEQ], F32, tag="dft")
                                nc.vector.tensor_copy(dft[:], field[:, c, :])
                                nc.sync.dma_start(
                                    dbg["dbg_field"][c * 128:(c + 1) * 128, :],
                                    dft[:])
                                dgt = p_dbg.tile([128, SEQ], F32, tag="dgt")
                                nc.vector.tensor_copy(dgt[:], gateT[:, c, :])
                                nc.sync.dma_start(
                                    dbg["dbg_gate"][c * 128:(c + 1) * 128, :],
                                    dgt[:])

                    # ---- E: coupling + gate multiply -> pgT_dram ----
                    with tc.tile_pool(name="p_ew", bufs=2) as p_ew:
                        for co in range(KC):
                            mf = p_ew.tile([128, KC, 128], F32, tag="mc_f")
                            nc.sync.dma_start(
                                mf[:],
                                Mcoup[:, co * 128:(co + 1) * 128]
                                .rearrange("(kc p) m -> p kc m", p=128))
                            mr = p_ew.tile([128, KC, 128], F32R, tag="mc_r")
                            nc.vector.tensor_copy(mr[:], mf[:])
                            for sb in range(4):
                                psc2 = psp.tile([128, 512], F32, tag="ps")
                                for ci in range(KC):
                                    nc.tensor.matmul(
                                        psc2[:], mr[:, ci, :],
                                        field[:, ci, sb * 512:(sb + 1) * 512],
                                        start=(ci == 0), stop=(ci == KC - 1))
                                pgs = p_ew.tile([128, 512], F32R, tag="pgs")
                                nc.vector.tensor_mul(
                                    pgs[:], psc2[:],
                                    gateT[:, co, sb * 512:(sb + 1) * 512])
                                nc.sync.dma_start(
                                    pgT_dram[co * 128:(co + 1) * 128,
                                             sb * 512:(sb + 1) * 512], pgs[:])

            # ================= phase F: final projection =================
            with (
                tc.tile_pool(name="p_wo", bufs=1) as p_wo,
                tc.tile_pool(name="p_fw", bufs=2) as p_fw,
            ):
                wo_r = p_wo.tile([128, KC, D], F32R, tag="wo_r")
                for k in range(KC):
                    wof = p_fw.tile([128, D], F32, tag="wof")
                    nc.sync.dma_start(wof[:], Wout[k * 128:(k + 1) * 128, :])
                    nc.vector.tensor_copy(wo_r[:, k, :], wof[:])
                bout_t = p_wo.tile([128, D], F32, tag="bout_t")
                nc.sync.dma_start(bout_t[:], boutB[:])
                for st in range(SEQ // 128):
                    pgt = p_fw.tile([128, KC, 128], F32R, tag="pgt")
                    nc.sync.dma_start(
                        pgt[:],
                        pgT_dram[:, st * 128:(st + 1) * 128]
                        .rearrange("(kc p) m -> p kc m", p=128))
                    outb = p_fw.tile([128, D], F32, tag="outb")
                    for cb in range(2):
                        pso = psp.tile([128, 512], F32, tag="ps")
                        for k in range(KC):
                            nc.tensor.matmul(pso[:], pgt[:, k, :],
                                             wo_r[:, k, cb * 512:(cb + 1) * 512],
                                             start=(k == 0), stop=(k == KC - 1))
                        nc.vector.tensor_add(outb[:, cb * 512:(cb + 1) * 512],
                                             pso[:],
                                             bout_t[:, cb * 512:(cb + 1) * 512])
                    nc.sync.dma_start(out[st * 128:(st + 1) * 128, :], outb[:])

    nc.compile()
    _PROGRAM_CACHE[key] = nc
    return nc


def _softmax(a, axis):
    a = a - a.max(axis=axis, keepdims=True)
    e = np.exp(a)
    return e / e.sum(axis=axis, keepdims=True)


def _host_prep(inputs):
    """Build per-core and replicated input tensors from full inputs."""
    x = np.asarray(inputs["x"], np.float32)
    Wqkv = np.ascontiguousarray(np.asarray(inputs["Wqkv"], np.float32))
    bqkv = np.asarray(inputs["bqkv"], np.float32)
    Wout = np.ascontiguousarray(np.asarray(inputs["Wout"], np.float32))
    bout = np.asarray(inputs["bout"], np.float32)
    Wgate = np.ascontiguousarray(np.asarray(inputs["Wgate"], np.float32))
    bgate = np.asarray(inputs["bgate"], np.float32)
    scale_gain = np.asarray(inputs["scale_gain"], np.float64)
    skip_w = np.asarray(inputs["skip_w"], np.float64)
    coupling = np.asarray(inputs["coupling"], np.float64)

    gains = _softmax(scale_gain, axis=0)              # [11, H]
    sw = 1.0 / (1.0 + np.exp(-skip_w))                # [2]
    coup = _softmax(coupling, axis=-1)                # [H, H]

    sidx = {s: i for i, s in enumerate(SHIFTS)}
    wtab = np.zeros((NT, H), np.float64)
    for j in range(N_SCALES):
        d = 1 << j
        for t in range(4):
            wtab[sidx[(3 - t) * d]] += D4[t] * gains[j]
    ch = np.arange(D)
    wchan = np.zeros((128, KC, NT), np.float32)
    for c in range(KC):
        heads = (ch[c * 128:(c + 1) * 128] // HD)
        wchan[:, c, :] = wtab[:, heads].T.astype(np.float32)

    Mc = np.zeros((D, D), np.float32)
    idx = np.arange(HD)
    for i in range(H):
        for j in range(H):
            Mc[j * HD + idx, i * HD + idx] = coup[i, j]

    bqkvT = bqkv.reshape(24, 128).T.copy()            # [128, 24]
    bgateT = bgate.reshape(8, 128).T.copy()           # [128, 8]
    boutB = np.broadcast_to(bout, (128, D)).copy()
    swt = np.broadcast_to(sw.astype(np.float32), (128, 2)).copy()
    bo = np.zeros((128, 2), np.float32)
    bo[0:64, 0] = 1.0
    bo[64:128, 1] = 1.0
    on = np.zeros((2, 128), np.float32)
    on[0, 0:64] = 1.0
    on[1, 64:128] = 1.0

    shared = dict(Wqkv=Wqkv, bqkvT=bqkvT, Wgate=Wgate, bgateT=bgateT,
                  Wout=Wout, boutB=boutB, Mcoup=Mc, wchan=wchan, swt=swt,
                  bo_in=bo, on_in=on)
    in_maps = []
    for c in range(NCORES):
        b, half = c // 2, c % 2
        g0 = half * SEQ
        xTc = np.ascontiguousarray(x[b, g0:g0 + SEQ, :].T)
        m = np.full((128, 1), float(half), np.float32)
        in_maps.append(dict(xT=xTc, mask=m, **shared))
    return in_maps


def run_cores(inputs, debug_outputs=False, trace=False):
    nc = _build_program(debug_outputs=debug_outputs)
    in_maps = _host_prep(inputs)
    res = run_bass_kernel_spmd(nc, in_maps, list(range(NCORES)), trace=trace)
    return res


def kernel(**inputs) -> np.ndarray:
    res = run_cores(inputs)
    out = np.empty((B, N, D), np.float32)
    for c in range(NCORES):
        b, half = c // 2, c % 2
        out[b, half * SEQ:(half + 1) * SEQ, :] = res.results[c]["out"]
    return out


# revision 20
# speedup vs baseline: 1.3536x; 1.0493x over previous
"""Trainium2 Bass kernel for CausalWaveletFieldAttention.

Shapes (hardcoded): x [B=4, N=4096, D=1024], H=16 heads, HD=64.
Sharding over 8 cores: core c handles (batch b = c//2, half = c%2), i.e.
2048 contiguous sequence rows of one batch.

Per-core pipeline (all big compute in "transposed" [channel, seq] layout):
  1. qkv^T = Wqkv-chunks (stationary) x x^T (moving)  -> [3072, 2048]
  2. k_mag per head via PE block-ones reduction of k^2, f0 = v * k_mag
  3. pairwise AllGather of f0 between the two halves of each batch
     (odd cores need the even core's f0 as causal-conv history; even
     cores multiply the gathered buffer by mask=0)
  4. 24-tap dilated wavelet FIR (softmax gains folded host-side) as PE
     diagonal-matmul PSUM accumulation over shifted slices, extended
     1024 rows back so the d=512/1024 skip taps can be applied locally
  5. skip taps via fused scalar_tensor_tensor on DVE
  6. head coupling as a dense [1024,1024] matmul with (coup^T (x) I_64)
  7. gate = sigmoid(q @ Wgate + b) fused into PSUM eviction on ScalarE
  8. out = (field*gate) @ Wout + bout, stored in natural [seq, ch] layout

All matmuls run in float32r (TF32-like, 4x faster than fp32 on TRN2 PE,
~1.4e-4 rel err per matmul).
"""

import numpy as np

import concourse.bass as bass
import concourse.mybir as mybir
import concourse.tile as tile
from concourse import bacc
from concourse.bass_utils import run_bass_kernel_spmd
from concourse.masks import make_identity

F32 = mybir.dt.float32
F32R = mybir.dt.float32r
AF = mybir.ActivationFunctionType

B, N, D, H, HD = 4, 4096, 1024, 16, 64
NCORES = 8
SEQ = N // 2          # 2048 rows per core
KC = D // 128         # 8 contraction chunks
CONVN = SEQ + 1024    # 3072 conv outputs (1024 extra for skip taps)
EXT = SEQ + 4096      # 6144 extended f0 buffer
D4 = [0.4829629131445341, 0.8365163037378079, 0.2241438680420134, -0.1294095225512604]
N_SCALES = 11
SPARSE_DILATIONS = (512, 1024)
SHIFTS = [0, 1, 2, 3, 4, 6, 8, 12, 16, 24, 32, 48, 64, 96, 128, 192, 256,
          384, 512, 768, 1024, 1536, 2048, 3072]
NT = len(SHIFTS)      # 24 taps

_PROGRAM_CACHE = {}


def _build_program(debug_outputs=False):
    key = bool(debug_outputs)
    if key in _PROGRAM_CACHE:
        return _PROGRAM_CACHE[key]

    nc = bacc.Bacc("TRN2", target_bir_lowering=False, debug=False,
                   num_devices=NCORES)

    # ---- parameters (per-core) ----
    xT = nc.declare_dram_parameter("xT", [D, SEQ], F32, isOutput=False)
    mask = nc.declare_dram_parameter("mask", [128, 1], F32, isOutput=False)
    Wqkv = nc.declare_dram_parameter("Wqkv", [D, 3 * D], F32, isOutput=False)
    bqkvT = nc.declare_dram_parameter("bqkvT", [128, 24], F32, isOutput=False)
    Wgate = nc.declare_dram_parameter("Wgate", [D, D], F32, isOutput=False)
    bgateT = nc.declare_dram_parameter("bgateT", [128, 8], F32, isOutput=False)
    Wout = nc.declare_dram_parameter("Wout", [D, D], F32, isOutput=False)
    boutB = nc.declare_dram_parameter("boutB", [128, D], F32, isOutput=False)
    Mcoup = nc.declare_dram_parameter("Mcoup", [D, D], F32, isOutput=False)
    wchan = nc.declare_dram_parameter("wchan", [128, KC, NT], F32, isOutput=False)
    swt = nc.declare_dram_parameter("swt", [128, 2], F32, isOutput=False)
    bo_in = nc.declare_dram_parameter("bo_in", [128, 2], F32, isOutput=False)
    on_in = nc.declare_dram_parameter("on_in", [2, 128], F32, isOutput=False)
    out = nc.declare_dram_parameter("out", [SEQ, D], F32, isOutput=True)

    dbg = {}
    if debug_outputs:
        for name, shape, dt in (("dbg_f0", [D, SEQ], F32R),
                                ("dbg_conv", [D, CONVN], F32),
                                ("dbg_field", [D, SEQ], F32),
                                ("dbg_gate", [D, SEQ], F32)):
            dbg[name] = nc.declare_dram_parameter(name, shape, dt, isOutput=True)

    # ---- internal DRAM ----
    f0_dram = [nc.dram_tensor(f"f0_dram{c}", [128, SEQ], F32R)
               for c in range(KC)]
    f0_gath = [nc.dram_tensor(f"f0_gath{c}", [2, 128, SEQ], F32R)
               for c in range(KC)]
    qT_dram = nc.dram_tensor("qT_dram", [D, SEQ], F32R)
    pgT_dram = nc.dram_tensor("pgT_dram", [D, SEQ], F32R)

    with tile.TileContext(nc) as tc:
        with (
            tc.tile_pool(name="psum", bufs=6, space="PSUM") as psp,
            tc.tile_pool(name="psum2", bufs=2, space="PSUM") as psp2,
            tc.tile_pool(name="const", bufs=1) as constp,
        ):
            # ---- constants ----
            ident = constp.tile([128, 128], F32)
            make_identity(nc, ident[:])
            bo_f = constp.tile([128, 2], F32)
            nc.sync.dma_start(bo_f[:], bo_in[:])
            bo_r = constp.tile([128, 2], F32R)
            nc.vector.tensor_copy(bo_r[:], bo_f[:])
            on_f = constp.tile([2, 128], F32)
            nc.sync.dma_start(on_f[:], on_in[:])
            on_r = constp.tile([2, 128], F32R)
            nc.vector.tensor_copy(on_r[:], on_f[:])

            wchan_t = constp.tile([128, KC, NT], F32)
            nc.sync.dma_start(wchan_t[:], wchan[:])
            swt_t = constp.tile([128, 2], F32)
            nc.sync.dma_start(swt_t[:], swt[:])
            bqkv_t = constp.tile([128, 24], F32)
            nc.sync.dma_start(bqkv_t[:], bqkvT[:])
            bgate_t = constp.tile([128, 8], F32)
            nc.sync.dma_start(bgate_t[:], bgateT[:])
            mask_t = constp.tile([128, 1], F32)
            nc.sync.dma_start(mask_t[:], mask[:])

            # ================= phase B/B': qkv =================
            with (
                tc.tile_pool(name="p_xm", bufs=1) as p_xm,
                tc.tile_pool(name="p_bw", bufs=2) as p_bw,
            ):
                xm_r = p_xm.tile([128, KC, SEQ], F32R, tag="xm_r")
                for k in range(KC):
                    xtmp = p_bw.tile([128, SEQ], F32, tag="xtmp")
                    nc.sync.dma_start(xtmp[:], xT[k * 128:(k + 1) * 128, :])
                    nc.vector.tensor_copy(xm_r[:, k, :], xtmp[:])

                def load_strip(col0):
                    sf = p_bw.tile([128, KC, 128], F32, tag="strip_f")
                    nc.sync.dma_start(
                        sf[:],
                        Wqkv[:, col0 * 128:(col0 + 1) * 128]
                        .rearrange("(kc p) m -> p kc m", p=128))
                    sr = p_bw.tile([128, KC, 128], F32R, tag="strip_r")
                    nc.vector.tensor_copy(sr[:], sf[:])
                    return sr

                def qkv_psum(strip, rb, tag="ps"):
                    ps = psp.tile([128, 512], F32, tag=tag)
                    for k in range(KC):
                        nc.tensor.matmul(ps[:], strip[:, k, :],
                                         xm_r[:, k, rb * 512:(rb + 1) * 512],
                                         start=(k == 0), stop=(k == KC - 1))
                    return ps

                for c in range(KC):
                    ks = load_strip(8 + c)
                    k2b = p_bw.tile([128, SEQ], F32R, tag="k2b")
                    for rb in range(4):
                        ps = qkv_psum(ks, rb)
                        nc.scalar.activation(k2b[:, rb * 512:(rb + 1) * 512],
                                             ps[:], AF.Square,
                                             bias=bqkv_t[:, 8 + c:9 + c])
                    vs = load_strip(16 + c)
                    vTb = p_bw.tile([128, SEQ], F32, tag="vTb")
                    for rb in range(4):
                        ps = qkv_psum(vs, rb)
                        nc.scalar.activation(vTb[:, rb * 512:(rb + 1) * 512],
                                             ps[:], AF.Identity,
                                             bias=bqkv_t[:, 16 + c:17 + c])
                    km = p_bw.tile([2, SEQ], F32R, tag="km")
                    for sb in range(4):
                        pss = psp2.tile([2, 512], F32, tag="ps2")
                        nc.tensor.matmul(pss[:], bo_r[:],
                                         k2b[:, sb * 512:(sb + 1) * 512],
                                         start=True, stop=True)
                        nc.scalar.activation(km[:, sb * 512:(sb + 1) * 512],
                                             pss[:], AF.Sqrt)
                    f0b = p_bw.tile([128, SEQ], F32R, tag="f0b")
                    for sb in range(4):
                        pse = psp.tile([128, 512], F32, tag="ps")
                        nc.tensor.matmul(pse[:], on_r[:],
                                         km[:, sb * 512:(sb + 1) * 512],
                                         start=True, stop=True)
                        nc.vector.tensor_mul(f0b[:, sb * 512:(sb + 1) * 512],
                                             vTb[:, sb * 512:(sb + 1) * 512],
                                             pse[:])
                    nc.sync.dma_start(f0_dram[c][:], f0b[:])
                    # pipelined pairwise f0 exchange (per chunk, overlaps B)
                    nc.gpsimd.collective_compute(
                        "AllGather", mybir.AluOpType.bypass,
                        replica_groups=[[0, 1], [2, 3], [4, 5], [6, 7]],
                        ins=[f0_dram[c][:]], outs=[f0_gath[c][:]])

                # q columns -> qT_dram
                for o in range(8):
                    qs = load_strip(o)
                    for rb in range(4):
                        ps = qkv_psum(qs, rb)
                        qst = p_bw.tile([128, 512], F32R, tag="qst")
                        nc.vector.tensor_scalar_add(qst[:], ps[:],
                                                    bqkv_t[:, o:o + 1])
                        nc.sync.dma_start(
                            qT_dram[o * 128:(o + 1) * 128,
                                    rb * 512:(rb + 1) * 512], qst[:])

            if debug_outputs:
                for c in range(KC):
                    nc.sync.dma_start(
                        dbg["dbg_f0"][c * 128:(c + 1) * 128, :], f0_dram[c][:])

            # ================= phase C..E =================
            with tc.tile_pool(name="p_field", bufs=1) as p_field:
                field = p_field.tile([128, KC, SEQ], F32R, tag="field")

                # ---- conv + skip per chunk ----
                # 4 largest taps run on DVE as fused shifted mul-adds (free-
                # dim shifts are free in this layout); the other 20 accumulate
                # on the PE as diagonal matmuls; skips run on GpSimd.
                DVE_TAPS = [1024, 1536, 2048, 3072]
                PE_TAPS = [(si, s) for si, s in enumerate(SHIFTS)
                           if s not in DVE_TAPS]
                with (
                    tc.tile_pool(name="p_cw", bufs=2) as p_cw,
                    tc.tile_pool(name="p_cw1", bufs=1) as p_cw1,
                ):
                    zc = p_cw1.tile([128, 512], F32, tag="zc")
                    nc.vector.memset(zc[:], 0.0)
                    exts = []
                    for i in range(2):
                        e = p_cw1.tile([128, EXT], F32R, tag=f"ext{i}")
                        for z in range(4):
                            nc.vector.tensor_copy(e[:, z * 512:(z + 1) * 512],
                                                  zc[:])
                        exts.append(e)
                    da0 = p_cw1.tile([128, CONVN], F32, tag="da0")
                    da1 = p_cw1.tile([128, CONVN], F32, tag="da1")
                    da = [da0, da1]
                    wd = p_cw1.tile([128, NT, 128], F32R, tag="wd")
                    for c in range(KC):
                        ext = exts[c % 2]
                        halo = p_cw.tile([128, SEQ], F32R, tag="halo")
                        nc.sync.dma_start(halo[:], f0_gath[c][0, :, :])
                        nc.vector.tensor_scalar_mul(ext[:, 2048:4096], halo[:],
                                                    mask_t[:, 0:1])
                        nc.sync.dma_start(ext[:, 4096:EXT], f0_dram[c][:])

                        for si, s in PE_TAPS:
                            nc.vector.tensor_scalar_mul(
                                wd[:, si, :], ident[:],
                                wchan_t[:, c, si:si + 1])
                        # DVE taps: ping-pong accumulate over full conv width
                        cur = None
                        for ti, s in enumerate(DVE_TAPS):
                            si = SHIFTS.index(s)
                            src = ext[:, 3072 - s:3072 - s + CONVN]
                            w = wchan_t[:, c, si:si + 1]
                            if cur is None:
                                cur = da[0]
                                nc.vector.tensor_scalar_mul(cur[:], src, w)
                            else:
                                nxt = da[ti % 2]
                                nc.vector.scalar_tensor_tensor(
                                    nxt[:], src, w, cur[:],
                                    op0=mybir.AluOpType.mult,
                                    op1=mybir.AluOpType.add)
                                cur = nxt
                        convb = p_cw.tile([128, CONVN], F32, tag="convb")
                        for ob in range(CONVN // 512):
                            psc = psp.tile([128, 512], F32, tag="ps")
                            first = PE_TAPS[0][0]
                            last = PE_TAPS[-1][0]
                            for si, s in PE_TAPS:
                                off = 3072 + ob * 512 - s
                                nc.tensor.matmul(psc[:], wd[:, si, :],
                                                 ext[:, off:off + 512],
                                                 start=(si == first),
                                                 stop=(si == last))
                            nc.vector.tensor_add(
                                convb[:, ob * 512:(ob + 1) * 512], psc[:],
                                cur[:, ob * 512:(ob + 1) * 512])
                        tmp = p_cw1.tile([128, SEQ], F32, tag="skiptmp")
                        nc.vector.scalar_tensor_tensor(
                            tmp[:], convb[:, 512:512 + SEQ], swt_t[:, 0:1],
                            convb[:, 1024:1024 + SEQ],
                            op0=mybir.AluOpType.mult, op1=mybir.AluOpType.add)
                        nc.vector.scalar_tensor_tensor(
                            field[:, c, :], convb[:, 0:SEQ], swt_t[:, 1:2],
                            tmp[:],
                            op0=mybir.AluOpType.mult, op1=mybir.AluOpType.add)
                        if debug_outputs:
                            nc.sync.dma_start(
                                dbg["dbg_conv"][c * 128:(c + 1) * 128, :],
                                convb[:])

                # ---- gate (D) then coupling+mult (E) ----
                with tc.tile_pool(name="p_gate", bufs=1) as p_gate:
                    gateT = p_gate.tile([128, 8, SEQ], F32, tag="gateT")
                    with (
                        tc.tile_pool(name="p_wg", bufs=1) as p_wg,
                        tc.tile_pool(name="p_qrb", bufs=2) as p_qrb,
                    ):
                        wg_r = p_wg.tile([128, KC, D], F32R, tag="wg_r")
                        for k in range(KC):
                            wgf = p_qrb.tile([128, D], F32, tag="wgf")
                            nc.sync.dma_start(wgf[:],
                                              Wgate[k * 128:(k + 1) * 128, :])
                            nc.vector.tensor_copy(wg_r[:, k, :], wgf[:])
                        for rb in range(4):
                            qrb = p_qrb.tile([128, KC, 512], F32R, tag="qrb")
                            nc.sync.dma_start(
                                qrb[:],
                                qT_dram[:, rb * 512:(rb + 1) * 512]
                                .rearrange("(kc p) n -> p kc n", p=128))
                            for gc in range(8):
                                psg = psp.tile([128, 512], F32, tag="ps")
                                for k in range(KC):
                                    nc.tensor.matmul(
                                        psg[:],
                                        wg_r[:, k, gc * 128:(gc + 1) * 128],
                                        qrb[:, k, :],
                                        start=(k == 0), stop=(k == KC - 1))
                                nc.scalar.activation(
                                    gateT[:, gc, rb * 512:(rb + 1) * 512],
                                    psg[:], AF.Sigmoid,
                                    bias=bgate_t[:, gc:gc + 1])

                    if debug_outputs:
                        with tc.tile_pool(name="p_dbg", bufs=2) as p_dbg:
                            for c in range(KC):
                                dft = p_dbg.tile([128, SEQ], F32, tag="dft")
                                nc.vector.tensor_copy(dft[:], field[:, c, :])
                                nc.sync.dma_start(
                                    dbg["dbg_field"][c * 128:(c + 1) * 128, :],
                                    dft[:])
                                dgt = p_dbg.tile([128, SEQ], F32, tag="dgt")
                                nc.vector.tensor_copy(dgt[:], gateT[:, c, :])
                                nc.sync.dma_start(
                                    dbg["dbg_gate"][c * 128:(c + 1) * 128, :],
                                    dgt[:])

                    # ---- E: coupling + gate multiply -> pgT_dram ----
                    with tc.tile_pool(name="p_ew", bufs=2) as p_ew:
                        for co in range(KC):
                            mf = p_ew.tile([128, KC, 128], F32, tag="mc_f")
                            nc.sync.dma_start(
                                mf[:],
                                Mcoup[:, co * 128:(co + 1) * 128]
                                .rearrange("(kc p) m -> p kc m", p=128))
                            mr = p_ew.tile([128, KC, 128], F32R, tag="mc_r")
                            nc.vector.tensor_copy(mr[:], mf[:])
                            for sb in range(4):
                                psc2 = psp.tile([128, 512], F32, tag="ps")
                                for ci in range(KC):
                                    nc.tensor.matmul(
                                        psc2[:], mr[:, ci, :],
                                        field[:, ci, sb * 512:(sb + 1) * 512],
                                        start=(ci == 0), stop=(ci == KC - 1))
                                pgs = p_ew.tile([128, 512], F32R, tag="pgs")
                                nc.vector.tensor_mul(
                                    pgs[:], psc2[:],
                                    gateT[:, co, sb * 512:(sb + 1) * 512])
                                nc.sync.dma_start(
                                    pgT_dram[co * 128:(co + 1) * 128,
                                             sb * 512:(sb + 1) * 512], pgs[:])

            # ================= phase F: final projection =================
            with (
                tc.tile_pool(name="p_wo", bufs=1) as p_wo,
                tc.tile_pool(name="p_fw", bufs=2) as p_fw,
            ):
                wo_r = p_wo.tile([128, KC, D], F32R, tag="wo_r")
                for k in range(KC):
                    wof = p_fw.tile([128, D], F32, tag="wof")
                    nc.sync.dma_start(wof[:], Wout[k * 128:(k + 1) * 128, :])
                    nc.vector.tensor_copy(wo_r[:, k, :], wof[:])
                bout_t = p_wo.tile([128, D], F32, tag="bout_t")
                nc.sync.dma_start(bout_t[:], boutB[:])
                for st in range(SEQ // 128):
                    pgt = p_fw.tile([128, KC, 128], F32R, tag="pgt")
                    nc.sync.dma_start(
                        pgt[:],
                        pgT_dram[:, st * 128:(st + 1) * 128]
                        .rearrange("(kc p) m -> p kc m", p=128))
                    outb = p_fw.tile([128, D], F32, tag="outb")
                    for cb in range(2):
                        pso = psp.tile([128, 512], F32, tag="ps")
                        for k in range(KC):
                            nc.tensor.matmul(pso[:], pgt[:, k, :],
                                             wo_r[:, k, cb * 512:(cb + 1) * 512],
                                             start=(k == 0), stop=(k == KC - 1))
                        nc.vector.tensor_add(outb[:, cb * 512:(cb + 1) * 512],
                                             pso[:],
                                             bout_t[:, cb * 512:(cb + 1) * 512])
                    nc.sync.dma_start(out[st * 128:(st + 1) * 128, :], outb[:])

    nc.compile()
    _PROGRAM_CACHE[key] = nc
    return nc


def _softmax(a, axis):
    a = a - a.max(axis=axis, keepdims=True)
    e = np.exp(a)
    return e / e.sum(axis=axis, keepdims=True)


def _host_prep(inputs):
    """Build per-core and replicated input tensors from full inputs."""
    x = np.asarray(inputs["x"], np.float32)
    Wqkv = np.ascontiguousarray(np.asarray(inputs["Wqkv"], np.float32))
    bqkv = np.asarray(inputs["bqkv"], np.float32)
    Wout = np.ascontiguousarray(np.asarray(inputs["Wout"], np.float32))
    bout = np.asarray(inputs["bout"], np.float32)
    Wgate = np.ascontiguousarray(np.asarray(inputs["Wgate"], np.float32))
    bgate = np.asarray(inputs["bgate"], np.float32)
    scale_gain = np.asarray(inputs["scale_gain"], np.float64)
    skip_w = np.asarray(inputs["skip_w"], np.float64)
    coupling = np.asarray(inputs["coupling"], np.float64)

    gains = _softmax(scale_gain, axis=0)              # [11, H]
    sw = 1.0 / (1.0 + np.exp(-skip_w))                # [2]
    coup = _softmax(coupling, axis=-1)                # [H, H]

    sidx = {s: i for i, s in enumerate(SHIFTS)}
    wtab = np.zeros((NT, H), np.float64)
    for j in range(N_SCALES):
        d = 1 << j
        for t in range(4):
            wtab[sidx[(3 - t) * d]] += D4[t] * gains[j]
    ch = np.arange(D)
    wchan = np.zeros((128, KC, NT), np.float32)
    for c in range(KC):
        heads = (ch[c * 128:(c + 1) * 128] // HD)
        wchan[:, c, :] = wtab[:, heads].T.astype(np.float32)

    Mc = np.zeros((D, D), np.float32)
    idx = np.arange(HD)
    for i in range(H):
        for j in range(H):
            Mc[j * HD + idx, i * HD + idx] = coup[i, j]

    bqkvT = bqkv.reshape(24, 128).T.copy()            # [128, 24]
    bgateT = bgate.reshape(8, 128).T.copy()           # [128, 8]
    boutB = np.broadcast_to(bout, (128, D)).copy()
    swt = np.broadcast_to(sw.astype(np.float32), (128, 2)).copy()
    bo = np.zeros((128, 2), np.float32)
    bo[0:64, 0] = 1.0
    bo[64:128, 1] = 1.0
    on = np.zeros((2, 128), np.float32)
    on[0, 0:64] = 1.0
    on[1, 64:128] = 1.0

    shared = dict(Wqkv=Wqkv, bqkvT=bqkvT, Wgate=Wgate, bgateT=bgateT,
                  Wout=Wout, boutB=boutB, Mcoup=Mc, wchan=wchan, swt=swt,
                  bo_in=bo, on_in=on)
    in_maps = []
    for c in range(NCORES):
        b, half = c // 2, c % 2
        g0 = half * SEQ
        xTc = np.ascontiguousarray(x[b, g0:g0 + SEQ, :].T)
        m = np.full((128, 1), float(half), np.float32)
        in_maps.append(dict(xT=xTc, mask=m, **shared))
    return in_maps


def run_cores(inputs, debug_outputs=False, trace=False):
    nc = _build_program(debug_outputs=debug_outputs)
    in_maps = _host_prep(inputs)
    res = run_bass_kernel_spmd(nc, in_maps, list(range(NCORES)), trace=trace)
    return res


def kernel(**inputs) -> np.ndarray:
    res = run_cores(inputs)
    out = np.empty((B, N, D), np.float32)
    for c in range(NCORES):
        b, half = c // 2, c % 2
        out[b, half * SEQ:(half + 1) * SEQ, :] = res.results[c]["out"]
    return out
